# revision 13
# baseline (speedup 1.0000x reference)
"""Trainium2 Bass kernel: dual-softmax cross-attention bilinear forms.

Math (per batch b, a = corr[b] in [N, N], N = 3072):
    attn = exp(2a) * (1/rowsum_a) outer (1/colsum_a)
    fund1 = v1^T attn v1,  fund2^T = v2^T attn^T v2
Device computes, per core (4 batches x 2 row-halves = 8 cores), for its
half slab [NH=1536, N]:
    E1 = exp(a + B) fp16 on the scalar engine (a arrives as fp8e4m3 --
    the quantization noise washes out through the dual normalization).
    rowsum via the activation accumulator; colsum partials via
    ones^T @ E1 fp16 matmuls spread over the PE's 4 column groups.
    e2 = fp8(E1^2 * rinv_row) -- one DVE scalar_tensor_tensor per tile.
    X = e2^T @ vr -- fp8 DoubleRow matmuls. Each (m, ipair) is ONE wide
    matmul (rhs free 1024 -> out 512, exactly one psum bank), so the
    per-group cost is ld(128cyc) + mm(256cyc) and nothing else.
    pos^T: stationary = 16 pos columns of vr per ipair, moving = wide
    512-col e2 chunks -> [16, 512] psum slots (DR outputs must sit at
    partition 0 -- DR + tile_position offsets fail codegen). 2 slots
    per round; round 0 streams, rounds 1-2 replay post-stream.
Host finishes: colsum normalization + the small [N,262] bilinear GEMMs.

Schedule: the scalar exp stream paces (12 x ~3.1us); squares lag one
tile on the DVE. PSUM: 2 colsum banks, 2 pos banks, 4 X banks. RES=4
m-tiles accumulate in psum across the stream; the other 20 run full
6-ipair bursts post-stream at ~165ns/group on the hot PE. Exports
(psum -> sbuf fp16) split scalar/vector once the streams drain.
"""

import numpy as np

import concourse.tile as tile
from concourse import bacc, bass_utils, mybir

B, N, C = 4, 3072, 256
H, W = 48, 64
CP = C + 6          # 262
CX = 2 * C          # 512: [x1 256 | x2 256]
NH = N // 2         # 1536 rows per core
NT = NH // 128      # 12 row tiles per core
NP = NT // 2        # 6 DoubleRow ipairs
MT = N // 128       # 24 column tiles
CS_CHUNK = 512
NCS = N // CS_CHUNK  # 6 colsum psum chunks
NPC = N // 256       # 12 pos^T moving chunks
CVP = CX + 16        # 528: fp8 v row: [x1 256 | x2 256 | pos 6 | pad 10]
B_SHIFT = 2.875      # E1 = exp(a + B_SHIFT); constants cancel on host

RES = 4              # m-tiles kept psum-resident through the stream

FP32 = mybir.dt.float32
FP16 = mybir.dt.float16
FP8 = mybir.dt.float8e4
DR = mybir.MatmulPerfMode.DoubleRow
MUL = mybir.AluOpType.mult

TRACE = False
LAST_RESULT = None
_CACHED_NC = None


def _build_kernel():
    nc = bacc.Bacc("TRN2", target_bir_lowering=False, debug=False)
    a_in = nc.dram_tensor("a_half", [NH, N], FP8, kind="ExternalInput").ap()
    v_in = nc.dram_tensor("v_half", [128, NT * CVP], FP8, kind="ExternalInput").ap()
    x_out = nc.dram_tensor("x_out", [128, MT * CX], FP16, kind="ExternalOutput").ap()
    pos_out = nc.dram_tensor("pos_out", [16, 6, CS_CHUNK], FP32, kind="ExternalOutput").ap()
    cs_out = nc.dram_tensor("cs_out", [8, CS_CHUNK], FP32, kind="ExternalOutput").ap()

    with tile.TileContext(nc) as tc:
        _kernel_body(tc, a_in, v_in, x_out, pos_out, cs_out)
    nc.compile()
    return nc


def _pos_slot(ch):
    """pos^T wide chunk ch (of 6) -> (round, bank).

    Each [16, 512] output fills one bank at partition 0; 2 banks per
    round, 3 rounds. Round 0 streams with the pairs; rounds 1-2 replay
    from the retained e2 after the stream.
    """
    r, b = divmod(ch, 2)
    return r, b


def _kernel_body(tc, a_in, v_in, x_out, pos_out, cs_out):
    nc = tc.nc
    with (
        tc.tile_pool(name="singles", bufs=1) as singles,
        tc.tile_pool(name="a_pool", bufs=4) as a_pool,
        tc.tile_pool(name="e_pool", bufs=4) as e_pool,
        tc.tile_pool(name="cs_psum", bufs=1, space="PSUM") as cs_psum,
        tc.tile_pool(name="pos_psum", bufs=1, space="PSUM") as pos_psum,
        tc.tile_pool(name="x_psum", bufs=RES, space="PSUM") as x_psum,
    ):
        ones_t = singles.tile([128, 1], FP16)
        nc.vector.memset(ones_t, 1.0)
        bias_t = singles.tile([128, 1], FP32)
        nc.vector.memset(bias_t, B_SHIFT)

        # prefetch the exp table-set off the critical path
        dummy_t = singles.tile([128, 1], FP32)
        nc.scalar.activation(
            out=dummy_t, in_=bias_t, func=mybir.ActivationFunctionType.Exp
        )

        # vr = fp8(v) arrives pre-packed from the host:
        # [x1 256 | x2 256 | pos 6 | pad 10] per tile
        vr_all = singles.tile([128, NT, CVP], FP8)
        e2_all = singles.tile([128, NT, N], FP8)
        rowsum_all = singles.tile([128, NT + 1], FP32)
        rinv_all = singles.tile([128, NT], FP32)
        x_sb = singles.tile([128, MT, CX], FP16)
        pos_sb = singles.tile([16, 6, CS_CHUNK], FP32)
        cs_sb = singles.tile([128, 2, CS_CHUNK], FP32)

        # colsum: 6 chunks in 2 psum banks at partitions 0/32/64/96.
        # pos^T: 2 [16,512] slots (partition 0) per round. Pre-zeroed;
        # every matmul accumulates (start=False).
        cs_bank = [
            cs_psum.tile([128, CS_CHUNK], FP32, name=f"csb{t}", tag=f"csb{t}")
            for t in range(2)
        ]
        pos_bank = [
            pos_psum.tile([128, CS_CHUNK], FP32, name=f"posb{t}", tag=f"posb{t}")
            for t in range(2)
        ]
        for t in range(2):
            nc.vector.memset(cs_bank[t], 0.0)
            nc.vector.memset(pos_bank[t], 0.0)

        def cs_ap(j):
            t, p = divmod(j, 4)
            return cs_bank[t][32 * p : 32 * p + 1, :]

        def stream_tile(i, e_t, col_lo, col_hi, accum_col):
            """exp + rowsum-accum for columns [col_lo, col_hi) of tile i."""
            a_t = a_pool.tile([128, N], FP8, name="a_t", tag="a_t")
            nc.sync.dma_start(
                out=a_t[:, col_lo:col_hi],
                in_=a_in[i * 128 : (i + 1) * 128, col_lo:col_hi],
            )
            if i == 0 and col_lo == 0:
                # fp8 v load rides behind the first a-chunk
                nc.sync.dma_start(out=vr_all, in_=v_in)
            nc.scalar.activation(
                out=e_t[:, col_lo:col_hi],
                in_=a_t[:, col_lo:col_hi],
                func=mybir.ActivationFunctionType.Exp,
                bias=bias_t,
                scale=1.0,
                accum_out=rowsum_all[:, accum_col : accum_col + 1],
            )
            for j in range(col_lo // CS_CHUNK, col_hi // CS_CHUNK):
                nc.tensor.matmul(
                    cs_ap(j),
                    lhsT=ones_t,
                    rhs=e_t[:, j * CS_CHUNK : (j + 1) * CS_CHUNK],
                    start=False,
                    stop=(i == NT - 1),
                    skip_group_check=True,
                    tile_position=(0, 32 * (j % 4)),
                )
            return a_t

        def square_tile(i, e_t):
            nc.vector.reciprocal(rinv_all[:, i : i + 1], rowsum_all[:, i : i + 1])
            nc.vector.scalar_tensor_tensor(
                out=e2_all[:, i, :],
                in0=e_t,
                scalar=rinv_all[:, i : i + 1],
                in1=e_t,
                op0=MUL,
                op1=MUL,
            )

        def gemm_pair(m, p, xp, first, last):
            """ipair p of m-tile's X: one wide DR matmul (out 512)."""
            nc.tensor.matmul(
                xp,
                lhsT=e2_all[:, 2 * p : 2 * p + 2, m * 128 : (m + 1) * 128],
                rhs=vr_all[:, 2 * p : 2 * p + 2, 0:CX],
                start=first,
                stop=last,
                perf_mode=DR,
                skip_group_check=True,
            )

        def pos_gemm(p, rnd):
            """pos^T += vr_pos_pair^T @ e2_pair for round rnd's 2 chunks."""
            lhsT = vr_all[:, 2 * p : 2 * p + 2, CX : CX + 16]
            for ch in range(2 * rnd, 2 * rnd + 2):
                _, t = _pos_slot(ch)
                nc.tensor.matmul(
                    pos_bank[t][0:16, :],
                    lhsT=lhsT,
                    rhs=e2_all[:, 2 * p : 2 * p + 2, ch * 512 : (ch + 1) * 512],
                    start=False,
                    stop=(p == NP - 1),
                    perf_mode=DR,
                    skip_group_check=True,
                )

        def pos_export(rnd):
            for t in range(2):
                nc.scalar.copy(
                    out=pos_sb[0:16, 2 * rnd + t, :], in_=pos_bank[t][0:16, :]
                )
                if rnd < 2:
                    nc.vector.memset(pos_bank[t], 0.0)

        def export(m, xp, eng):
            if eng == 0:
                nc.scalar.copy(out=x_sb[:, m, :], in_=xp)
            else:
                nc.vector.tensor_copy(out=x_sb[:, m, :], in_=xp)

        # ---- streaming phase ----------------------------------------
        res_xp = [
            x_psum.tile([128, CX], FP32, name="xp", tag="xp")
            for m in range(RES)
        ]
        for i in range(NT):
            e_t = e_pool.tile([128, N], FP16, name="e_t", tag="e_t")
            if i == 0:
                # split tile 0 so the first exp starts sooner
                stream_tile(0, e_t, 0, N // 2, NT)
                stream_tile(0, e_t, N // 2, N, 0)
                nc.vector.tensor_add(
                    rowsum_all[:, 0:1],
                    rowsum_all[:, 0:1],
                    rowsum_all[:, NT : NT + 1],
                )
            else:
                stream_tile(i, e_t, 0, N, i)
            square_tile(i, e_t)
            if i % 2 == 1:
                p = i // 2
                for m in range(RES):
                    gemm_pair(m, p, res_xp[m], first=(p == 0), last=(p == NP - 1))
                pos_gemm(p, 0)

        # ---- tail: resident exports + remaining m bursts --------------
        for m in range(RES):
            export(m, res_xp[m], m % 2)

        stored = 0

        def store_upto(hi):
            nonlocal stored
            while stored + 2 <= hi:
                g = stored // 2
                nc.sync.dma_start(
                    out=x_out[:, 2 * g * CX : 2 * (g + 1) * CX],
                    in_=x_sb[:, 2 * g : 2 * (g + 1), :],
                )
                stored += 2

        # colsum psum -> sbuf -> DRAM (4 used rows per bank)
        for t in range(2):
            nc.scalar.copy(out=cs_sb[:, t, :], in_=cs_bank[t])
            nc.sync.dma_start(
                out=cs_out[4 * t : 4 * t + 4, :], in_=cs_sb[0:128:32, t, :]
            )

        for idx, m in enumerate(range(RES, MT)):
            xp = x_psum.tile([128, CX], FP32, name="xp", tag="xp")
            for p in range(NP):
                gemm_pair(m, p, xp, first=(p == 0), last=(p == NP - 1))
            export(m, xp, m % 2)
            store_upto(m)
            # pos rounds 1-2 replay between the early tail bursts
            if idx == 1:
                pos_export(0)
            if idx == 3:
                for p in range(NP):
                    pos_gemm(p, 1)
            if idx == 6:
                pos_export(1)
            if idx == 8:
                for p in range(NP):
                    pos_gemm(p, 2)
            if idx == 11:
                pos_export(2)
                nc.sync.dma_start(out=pos_out, in_=pos_sb)
        store_upto(MT)


def _positional_encodings():
    ys = np.linspace(-1.0, 1.0, H, dtype=np.float32)
    xs = np.linspace(-1.0, 1.0, W, dtype=np.float32)
    p3 = np.tile(ys, W)
    p4 = np.repeat(xs, H)
    pos = np.stack([p3 * p3, p4 * p4, p3 * p4, p3, p4, np.ones_like(p3)], axis=-1)
    return pos.astype(np.float32)  # [N, 6]


def kernel(x1, x2, corr, W_proj, b_proj):
    global _CACHED_NC, LAST_RESULT
    x1 = np.asarray(x1, dtype=np.float32)
    x2 = np.asarray(x2, dtype=np.float32)
    corr = np.asarray(corr, dtype=np.float32)
    W_proj = np.asarray(W_proj, dtype=np.float32)
    b_proj = np.asarray(b_proj, dtype=np.float32)

    import ml_dtypes

    pos = _positional_encodings()
    a = corr.reshape(B, N, N).astype(ml_dtypes.float8_e4m3)
    # v = [x1 | x2 | pos | pad] quantized to fp8 on the host (vr = v verbatim
    # since the row normalization rides inside e2 on the device)
    v_all = np.zeros((B, N, CVP), dtype=np.float32)
    v_all[:, :, 0:C] = x1
    v_all[:, :, C : 2 * C] = x2
    v_all[:, :, CX : CX + 6] = np.broadcast_to(pos, (B, N, 6))
    v_all = v_all.astype(ml_dtypes.float8_e4m3)

    if _CACHED_NC is None:
        _CACHED_NC = _build_kernel()
    nc = _CACHED_NC

    in_maps = []
    for b in range(B):
        for h in range(2):
            rows = slice(h * NH, (h + 1) * NH)
            # pack v partition-major: v_packed[p, i*CVP + c] = v[i*128+p, c]
            vp = (
                v_all[b, rows, :]
                .reshape(NT, 128, CVP)
                .transpose(1, 0, 2)
                .reshape(128, NT * CVP)
            )
            in_maps.append(
                {
                    "a_half": np.ascontiguousarray(a[b, rows, :]),
                    "v_half": np.ascontiguousarray(vp),
                }
            )

    res = bass_utils.run_bass_kernel_spmd(
        nc, in_maps, core_ids=list(range(8)), trace=TRACE
    )
    LAST_RESULT = res

    v1 = np.concatenate([x1, np.broadcast_to(pos, (B, N, 6))], axis=2)
    v2 = np.concatenate([x2, np.broadcast_to(pos, (B, N, 6))], axis=2)

    out1 = np.empty((B, CP, C), dtype=np.float32)
    out2 = np.empty((B, CP, C), dtype=np.float32)
    for b in range(B):
        r0, r1 = res.results[2 * b], res.results[2 * b + 1]
        # unpack X: X[m*128+p, c] = x_out[p, m*CX + c]
        X = (
            r0["x_out"].astype(np.float32) + r1["x_out"].astype(np.float32)
        ).reshape(128, MT, CX).transpose(1, 0, 2).reshape(N, CX)
        # decode pos^T slots: posT[0:6, ch*512:(ch+1)*512] = slot ch
        pos_raw = r0["pos_out"] + r1["pos_out"]   # [16, 6, 512]
        posT = pos_raw[0:6].reshape(6, N)
        pos_x = posT.T                             # [N, 6]
        # colsum chunks: rows 0-3 = bank0 chunks 0-3, rows 4-5 = chunks 4-5
        colsum = np.empty(N, dtype=np.float32)
        for j in range(NCS):
            t, p = divmod(j, 4)
            colsum[j * CS_CHUNK : (j + 1) * CS_CHUNK] = (
                r0["cs_out"][4 * t + p] + r1["cs_out"][4 * t + p]
            )
        c = 1.0 / colsum
        vc1 = v1[b] * c[:, None]
        vc2 = v2[b] * c[:, None]
        X1 = np.concatenate([X[:, 0:256], pos_x], axis=1)   # [N, 262]
        X2 = np.concatenate([X[:, 256:512], pos_x], axis=1)
        fund1 = X1.T @ vc1      # [262, 262] = v1^T attn v1
        fund2t = X2.T @ vc2     # = (v2^T attn^T v2)^T
        out1[b] = fund1.T @ W_proj + b_proj
        out2[b] = fund2t @ W_proj + b_proj
    return (out2, out1)


# revision 14
# speedup vs baseline: 1.2011x; 1.2011x over previous
"""Trainium2 Bass kernel: dual-softmax cross-attention bilinear forms.

Math (per batch b, a = corr[b] in [N, N], N = 3072):
    attn = exp(2a) * (1/rowsum_a) outer (1/colsum_a)
    fund1 = v1^T attn v1,  fund2^T = v2^T attn^T v2
Device computes, per core (4 batches x 2 row-halves = 8 cores), for its
half slab [NH=1536, N]:
    E1 = exp(a + B) fp16 on the scalar engine (a arrives as fp8e4m3 --
    the quantization noise washes out through the dual normalization).
    rowsum via the activation accumulator; colsum partials via
    ones^T @ E1 fp16 matmuls spread over the PE's 4 column groups.
    e2 = fp8(E1^2 * rinv_row) -- one DVE scalar_tensor_tensor per tile.
    X = e2^T @ vr -- fp8 DoubleRow matmul trios per (m, ipair):
    [0:256 | 256:512 | pos 16-wide window in a shared bank]. All three
    share the same 256-row stationary; ~277ns/trio warm. (A single
    1024-wide moving matmul is NOT faster -- cost follows moving size --
    and its long SBUF bursts stall the scalar engine's exp reads.)
Host finishes: colsum normalization + the small [N,262] bilinear GEMMs.

Schedule: the scalar exp stream paces (12 x ~3.1us back to back, first
tile split in quarters for an early start; the v load is deferred so it
cannot delay exp #2). Squares lag one tile on the DVE. PSUM: 2 colsum
banks, 1 shared pos bank, 5 X banks. RES=5 m-tiles accumulate in psum
across the stream; the other 19 run dense 6-ipair bursts post-stream on
the hot PE. Exports (psum -> sbuf fp16) go to the scalar engine first
(it drains its exp backlog ~3us before the DVE finishes squares), then
alternate; X stores are grouped DMAs.
"""

import numpy as np

import concourse.tile as tile
from concourse import bacc, bass_utils, mybir

B, N, C = 4, 3072, 256
H, W = 48, 64
CP = C + 6          # 262
CX = 2 * C          # 512: [x1 256 | x2 256]
NH = N // 2         # 1536 rows per core
NT = NH // 128      # 12 row tiles per core
NP = NT // 2        # 6 DoubleRow ipairs
MT = N // 128       # 24 column tiles
CS_CHUNK = 512
NCS = N // CS_CHUNK  # 6 colsum psum chunks
CVP = CX + 16        # 528: fp8 v row: [x1 256 | x2 256 | pos 6 | pad 10]
B_SHIFT = 2.875      # E1 = exp(a + B_SHIFT); constants cancel on host

RES = 5              # m-tiles kept psum-resident through the stream
SC_EXPORTS = 8       # leading tail exports handled by the scalar engine

FP32 = mybir.dt.float32
FP16 = mybir.dt.float16
FP8 = mybir.dt.float8e4
DR = mybir.MatmulPerfMode.DoubleRow
MUL = mybir.AluOpType.mult

TRACE = False
LAST_RESULT = None
_CACHED_NC = None


def _build_kernel():
    nc = bacc.Bacc("TRN2", target_bir_lowering=False, debug=False)
    a_in = nc.dram_tensor("a_half", [NH, N], FP8, kind="ExternalInput").ap()
    v_in = nc.dram_tensor("v_half", [128, NT * CVP], FP8, kind="ExternalInput").ap()
    x_out = nc.dram_tensor("x_out", [128, MT * CX], FP16, kind="ExternalOutput").ap()
    pos_out = nc.dram_tensor("pos_out", [128, MT * 16], FP32, kind="ExternalOutput").ap()
    cs_out = nc.dram_tensor("cs_out", [8, CS_CHUNK], FP32, kind="ExternalOutput").ap()

    with tile.TileContext(nc) as tc:
        _kernel_body(tc, a_in, v_in, x_out, pos_out, cs_out)
    nc.compile()
    return nc


def _kernel_body(tc, a_in, v_in, x_out, pos_out, cs_out):
    nc = tc.nc
    with (
        tc.tile_pool(name="singles", bufs=1) as singles,
        tc.tile_pool(name="a_pool", bufs=4) as a_pool,
        tc.tile_pool(name="e_pool", bufs=4) as e_pool,
        tc.tile_pool(name="cs_psum", bufs=1, space="PSUM") as cs_psum,
        tc.tile_pool(name="pos_psum", bufs=1, space="PSUM") as pos_psum,
        tc.tile_pool(name="x_psum", bufs=RES, space="PSUM") as x_psum,
    ):
        ones_t = singles.tile([128, 1], FP16)
        nc.vector.memset(ones_t, 1.0)
        bias_t = singles.tile([128, 1], FP32)
        nc.vector.memset(bias_t, B_SHIFT)

        # prefetch the exp table-set off the critical path
        dummy_t = singles.tile([128, 1], FP32)
        nc.scalar.activation(
            out=dummy_t, in_=bias_t, func=mybir.ActivationFunctionType.Exp
        )

        # vr = fp8(v) arrives pre-packed from the host:
        # [x1 256 | x2 256 | pos 6 | pad 10] per tile
        vr_all = singles.tile([128, NT, CVP], FP8)
        e2_all = singles.tile([128, NT, N], FP8)
        rowsum_all = singles.tile([128, NT + 4], FP32)
        rinv_all = singles.tile([128, NT], FP32)
        x_sb = singles.tile([128, MT, CX], FP16)
        pos_sb = singles.tile([128, MT * 16], FP32)
        cs_sb = singles.tile([128, 2, CS_CHUNK], FP32)

        # colsum: 6 chunks in 2 psum banks at partitions 0/32/64/96.
        # pos: 16-wide window per m-tile, all in one shared bank.
        # Pre-zeroed; every matmul accumulates (start=False).
        cs_bank = [
            cs_psum.tile([128, CS_CHUNK], FP32, name=f"csb{t}", tag=f"csb{t}")
            for t in range(2)
        ]
        pos_bank = pos_psum.tile([128, CS_CHUNK], FP32, name="posb", tag="posb")
        for t in range(2):
            nc.vector.memset(cs_bank[t], 0.0)
        nc.vector.memset(pos_bank, 0.0)

        def cs_ap(j):
            t, p = divmod(j, 4)
            return cs_bank[t][32 * p : 32 * p + 1, :]

        def stream_chunk(i, e_t, col_lo, col_hi, accum_col):
            """DMA + exp + colsum for columns [col_lo, col_hi) of tile i."""
            a_t = a_pool.tile([128, N], FP8, name="a_t", tag="a_t")
            nc.sync.dma_start(
                out=a_t[:, col_lo:col_hi],
                in_=a_in[i * 128 : (i + 1) * 128, col_lo:col_hi],
            )
            nc.scalar.activation(
                out=e_t[:, col_lo:col_hi],
                in_=a_t[:, col_lo:col_hi],
                func=mybir.ActivationFunctionType.Exp,
                bias=bias_t,
                scale=1.0,
                accum_out=rowsum_all[:, accum_col : accum_col + 1],
            )
            for j in range(col_lo // CS_CHUNK, col_hi // CS_CHUNK):
                nc.tensor.matmul(
                    cs_ap(j),
                    lhsT=ones_t,
                    rhs=e_t[:, j * CS_CHUNK : (j + 1) * CS_CHUNK],
                    start=False,
                    stop=(i == NT - 1),
                    skip_group_check=True,
                    tile_position=(0, 32 * (j % 4)),
                )

        def square_tile(i, e_t):
            nc.vector.reciprocal(rinv_all[:, i : i + 1], rowsum_all[:, i : i + 1])
            nc.vector.scalar_tensor_tensor(
                out=e2_all[:, i, :],
                in0=e_t,
                scalar=rinv_all[:, i : i + 1],
                in1=e_t,
                op0=MUL,
                op1=MUL,
            )

        def gemm_trio(m, p, xp, first, last):
            """ipair p of m's X: three matmuls sharing one 256-row stationary."""
            lhsT = e2_all[:, 2 * p : 2 * p + 2, m * 128 : (m + 1) * 128]
            nc.tensor.matmul(
                xp[:, 0:256],
                lhsT=lhsT,
                rhs=vr_all[:, 2 * p : 2 * p + 2, 0:256],
                start=first,   # clears the bank
                stop=last,
                perf_mode=DR,
                skip_group_check=True,
            )
            nc.tensor.matmul(
                xp[:, 256:512],
                lhsT=lhsT,
                rhs=vr_all[:, 2 * p : 2 * p + 2, 256:512],
                start=False,   # bank cleared by the 0:256 start
                stop=last,
                perf_mode=DR,
                skip_group_check=True,
            )
            # pos columns: shared pre-zeroed bank, never start=True
            nc.tensor.matmul(
                pos_bank[:, 16 * m : 16 * (m + 1)],
                lhsT=lhsT,
                rhs=vr_all[:, 2 * p : 2 * p + 2, CX : CX + 16],
                start=False,
                stop=last,
                perf_mode=DR,
                skip_group_check=True,
            )

        def export(m, xp, eng):
            if eng == 0:
                nc.scalar.copy(out=x_sb[:, m, :], in_=xp)
            else:
                nc.vector.tensor_copy(out=x_sb[:, m, :], in_=xp)

        # ---- streaming phase ----------------------------------------
        res_xp = [
            x_psum.tile([128, CX], FP32, name="xp", tag="xp")
            for m in range(RES)
        ]
        for i in range(NT):
            e_t = e_pool.tile([128, N], FP16, name="e_t", tag="e_t")
            if i == 0:
                # tile 0 in quarters: the first exp starts ~3us sooner
                for q in range(4):
                    stream_chunk(0, e_t, q * (N // 4), (q + 1) * (N // 4),
                                 0 if q == 0 else NT + q - 1)
                for q in range(3):
                    nc.vector.tensor_add(
                        rowsum_all[:, 0:1],
                        rowsum_all[:, 0:1],
                        rowsum_all[:, NT + q : NT + q + 1],
                    )
                # v load rides after tile 0 so it cannot delay exp #2
                nc.sync.dma_start(out=vr_all, in_=v_in)
            elif i == 1:
                stream_chunk(1, e_t, 0, N // 2, 1)
                stream_chunk(1, e_t, N // 2, N, NT)
                nc.vector.tensor_add(
                    rowsum_all[:, 1:2],
                    rowsum_all[:, 1:2],
                    rowsum_all[:, NT : NT + 1],
                )
            else:
                stream_chunk(i, e_t, 0, N, i)
            square_tile(i, e_t)
            if i % 2 == 1:
                p = i // 2
                for m in range(RES):
                    gemm_trio(m, p, res_xp[m], first=(p == 0), last=(p == NP - 1))

        # ---- tail ------------------------------------------------------
        # colsum psum -> sbuf -> DRAM first: the scalar engine is free
        # ~3us before the DVE finishes the last squares
        for t in range(2):
            nc.scalar.copy(out=cs_sb[:, t, :], in_=cs_bank[t])
            nc.sync.dma_start(
                out=cs_out[4 * t : 4 * t + 4, :], in_=cs_sb[0:128:32, t, :]
            )
        for m in range(RES):
            export(m, res_xp[m], 0)

        stored = 0

        def store_upto(hi):
            nonlocal stored
            while stored + 2 <= hi:
                g = stored // 2
                nc.sync.dma_start(
                    out=x_out[:, 2 * g * CX : 2 * (g + 1) * CX],
                    in_=x_sb[:, 2 * g : 2 * (g + 1), :],
                )
                stored += 2

        for k, m in enumerate(range(RES, MT)):
            xp = x_psum.tile([128, CX], FP32, name="xp", tag="xp")
            for p in range(NP):
                gemm_trio(m, p, xp, first=(p == 0), last=(p == NP - 1))
            export(m, xp, 0 if k < SC_EXPORTS else (m % 2))
            store_upto(m)
        store_upto(MT)

        # pos bank: single export + store
        nc.vector.tensor_copy(out=pos_sb, in_=pos_bank[:, 0 : MT * 16])
        nc.sync.dma_start(out=pos_out, in_=pos_sb)


def _positional_encodings():
    ys = np.linspace(-1.0, 1.0, H, dtype=np.float32)
    xs = np.linspace(-1.0, 1.0, W, dtype=np.float32)
    p3 = np.tile(ys, W)
    p4 = np.repeat(xs, H)
    pos = np.stack([p3 * p3, p4 * p4, p3 * p4, p3, p4, np.ones_like(p3)], axis=-1)
    return pos.astype(np.float32)  # [N, 6]


def kernel(x1, x2, corr, W_proj, b_proj):
    global _CACHED_NC, LAST_RESULT
    x1 = np.asarray(x1, dtype=np.float32)
    x2 = np.asarray(x2, dtype=np.float32)
    corr = np.asarray(corr, dtype=np.float32)
    W_proj = np.asarray(W_proj, dtype=np.float32)
    b_proj = np.asarray(b_proj, dtype=np.float32)

    import ml_dtypes

    pos = _positional_encodings()
    a = corr.reshape(B, N, N).astype(ml_dtypes.float8_e4m3)
    # v = [x1 | x2 | pos | pad] quantized to fp8 on the host (vr = v verbatim
    # since the row normalization rides inside e2 on the device)
    v_all = np.zeros((B, N, CVP), dtype=np.float32)
    v_all[:, :, 0:C] = x1
    v_all[:, :, C : 2 * C] = x2
    v_all[:, :, CX : CX + 6] = np.broadcast_to(pos, (B, N, 6))
    v_all = v_all.astype(ml_dtypes.float8_e4m3)

    if _CACHED_NC is None:
        _CACHED_NC = _build_kernel()
    nc = _CACHED_NC

    in_maps = []
    for b in range(B):
        for h in range(2):
            rows = slice(h * NH, (h + 1) * NH)
            # pack v partition-major: v_packed[p, i*CVP + c] = v[i*128+p, c]
            vp = (
                v_all[b, rows, :]
                .reshape(NT, 128, CVP)
                .transpose(1, 0, 2)
                .reshape(128, NT * CVP)
            )
            in_maps.append(
                {
                    "a_half": np.ascontiguousarray(a[b, rows, :]),
                    "v_half": np.ascontiguousarray(vp),
                }
            )

    res = bass_utils.run_bass_kernel_spmd(
        nc, in_maps, core_ids=list(range(8)), trace=TRACE
    )
    LAST_RESULT = res

    v1 = np.concatenate([x1, np.broadcast_to(pos, (B, N, 6))], axis=2)
    v2 = np.concatenate([x2, np.broadcast_to(pos, (B, N, 6))], axis=2)

    out1 = np.empty((B, CP, C), dtype=np.float32)
    out2 = np.empty((B, CP, C), dtype=np.float32)
    for b in range(B):
        r0, r1 = res.results[2 * b], res.results[2 * b + 1]
        # unpack X: X[m*128+p, c] = x_out[p, m*CX + c]
        X = (
            r0["x_out"].astype(np.float32) + r1["x_out"].astype(np.float32)
        ).reshape(128, MT, CX).transpose(1, 0, 2).reshape(N, CX)
        # pos_x[m*128+p, k] = pos_out[p, 16*m + k], k < 6
        pos_raw = r0["pos_out"] + r1["pos_out"]   # [128, MT*16]
        pos_x = (
            pos_raw.reshape(128, MT, 16)[:, :, 0:6]
            .transpose(1, 0, 2)
            .reshape(N, 6)
        )
        # colsum chunks: rows 0-3 = bank0 chunks 0-3, rows 4-5 = chunks 4-5
        colsum = np.empty(N, dtype=np.float32)
        for j in range(NCS):
            t, p = divmod(j, 4)
            colsum[j * CS_CHUNK : (j + 1) * CS_CHUNK] = (
                r0["cs_out"][4 * t + p] + r1["cs_out"][4 * t + p]
            )
        c = 1.0 / colsum
        vc1 = v1[b] * c[:, None]
        vc2 = v2[b] * c[:, None]
        X1 = np.concatenate([X[:, 0:256], pos_x], axis=1)   # [N, 262]
        X2 = np.concatenate([X[:, 256:512], pos_x], axis=1)
        fund1 = X1.T @ vc1      # [262, 262] = v1^T attn v1
        fund2t = X2.T @ vc2     # = (v2^T attn^T v2)^T
        out1[b] = fund1.T @ W_proj + b_proj
        out2[b] = fund2t @ W_proj + b_proj
    return (out2, out1)
